# revision 6
# baseline (speedup 1.0000x reference)
"""ChebConv (K=3) kernel for Trainium2, data-parallel over batch across 8 NeuronCores.

Math (per batch b):
    d    = adj.sum(axis=1)  (row sums);  s = (d + 1e-6)^-0.5;  q = s^2
    M    = Diag(s) A Diag(s);  L = I - M
    Tx0 = x, Tx1 = x - Mx, Tx2 = 2(Tx1 - M Tx1) - x
    out  = relu(sum_k Txk @ W[k] + bsum)

Kernel-side reformulation (everything bf16 except PSUM accum):
    y0n[j,f]   = s[j] x[j,f]                      (natural node-partition tiles)
    ats2[j,i]  = A[i,j] q[i]                      (A transposed via PE matmuls with
                                                   rhs = diag(q), built per row-tile)
    w1T[f,i]   = sum_j y0n[j,f] ats2[j,i]         = s[i](Mx)[i,f]         (pass 1)
    y1T = y0T - w1T;  y1n = y0n - nat(w1T)        (plain subtract - q folded in)
    w2T[f,i]   = sum_j y1n[j,f] ats2[j,i]         = s[i](M Tx1)[i,f]      (pass 2)
    y2T = (2 y1T - y0T) - 2 w2T
    outT[fo,i] = sum_k sum_f W[k][f,fo] ykT[f,i]  (lhsT = W natural, rhs = ykT)
    out[i,fo]  = relu(dsq[i] * nat(outT) + bsum)  (dsq = (d+eps)^0.5)

Performance structure vs the naive version:
  - adj streams via gpsimd SWDGE with fp32->bf16 cast during DMA (8 x 2MB chunks);
    x/consts ride the scalar HWDGE ring; out stores ride sync.
  - PE transposes A per row-tile (rhs = diag(q)) and runs pass-1 triangularly
    inside the stream loop so the tensor engine never idles (HAM stays warm).
  - pass-2 runs per 512-column group with a pipelined epilogue (copy, y2T,
    output matmuls, transpose-back, scale+relu, store).
  - PSUM->SBUF copies are [128,512] ops alternating between DVE and ACT.
"""

import numpy as np

B, N, F, K = 8, 2048, 128, 3
P = 128
NT = N // P   # 16 node tiles
NCH = NT // 2  # 8 dma chunks of 2 tiles
EPS = 1e-6
NCORES = 8

_cache = {}


def _build_nc():
    from contextlib import ExitStack

    import concourse.bacc as bacc
    import concourse.tile as tile
    from concourse import mybir

    f32 = mybir.dt.float32
    bf16 = mybir.dt.bfloat16
    AF = mybir.ActivationFunctionType
    OP = mybir.AluOpType
    AX = mybir.AxisListType

    nc = bacc.Bacc("TRN2", target_bir_lowering=False, debug=False, num_devices=NCORES)
    adj = nc.dram_tensor("adj", [N, N], f32, kind="ExternalInput").ap()
    x = nc.dram_tensor("x", [N, F], f32, kind="ExternalInput").ap()
    W = nc.dram_tensor("W", [K, F, F], f32, kind="ExternalInput").ap()
    bsum_d = nc.dram_tensor("bsum", [P, F], f32, kind="ExternalInput").ap()
    ident = nc.dram_tensor("ident", [P, P], f32, kind="ExternalInput").ap()
    out = nc.dram_tensor("out", [N, F], f32, kind="ExternalOutput").ap()
    out_t = out.rearrange("(t p) f -> p t f", p=P)
    adj_c = adj.rearrange("(c t p) n -> p c t n", p=P, t=2)
    x_t = x.rearrange("(t p) f -> p t f", p=P)

    with ExitStack() as ctx:
        tc = ctx.enter_context(tile.TileContext(nc))
        consts = ctx.enter_context(tc.tile_pool(name="consts", bufs=1))
        ap_ = ctx.enter_context(tc.tile_pool(name="achunk", bufs=5))
        big = ctx.enter_context(tc.tile_pool(name="big", bufs=1))
        small = ctx.enter_context(tc.tile_pool(name="small", bufs=4))
        scr = ctx.enter_context(tc.tile_pool(name="scr", bufs=2))
        ps_acc = ctx.enter_context(tc.tile_pool(name="ps_acc", bufs=1, space="PSUM"))
        ps_t = ctx.enter_context(tc.tile_pool(name="ps_t", bufs=4, space="PSUM"))

        # ---- constants -------------------------------------------------
        ident_bf = consts.tile([P, P], bf16)
        nc.gpsimd.dma_start(out=ident_bf, in_=ident)
        w_bf = consts.tile([P, K, F], bf16)
        nc.gpsimd.dma_start(out=w_bf, in_=W.rearrange("k i o -> i k o"))
        bsum = consts.tile([P, F], f32)
        nc.scalar.dma_start(out=bsum, in_=bsum_d)
        x_f = consts.tile([P, NT, F], f32)
        nc.scalar.dma_start(out=x_f, in_=x_t)
        eps_sb = consts.tile([P, 1], f32)
        nc.vector.memset(eps_sb, EPS)

        x_bf = big.tile([P, NT, F], bf16)
        nc.vector.tensor_copy(out=x_bf, in_=x_f)

        # per-node scalars, [P, NT]: column r holds values for node tile r
        dsq = consts.tile([P, NT], f32)
        sinv = consts.tile([P, NT], f32)

        y0n = big.tile([P, NT, F], bf16)
        y1n = big.tile([P, NT, F], bf16)
        ats2 = big.tile([P, NT, N], bf16)  # [j_in_tile, c(j tile), i]: A[i,j]*q[i]
        y0T = big.tile([P, N], bf16)
        y1T = big.tile([P, N], bf16)
        ttT = big.tile([P, N], bf16)
        y2T = big.tile([P, N], bf16)
        w1bf = big.tile([P, N], bf16)
        w2bf = big.tile([P, N], bf16)
        oTbf = big.tile([P, N], bf16)

        z1 = ps_acc.tile([P, N], f32, tag="acc")

        cp = {"i": 0}

        def alt_copy(out, in_):
            cp["i"] += 1
            if cp["i"] % 2:
                nc.vector.tensor_copy(out=out, in_=in_)
            else:
                nc.scalar.copy(out=out, in_=in_)

        # ---- streaming phase: cast-DMA, rowsum, transpose, triangular
        #      pass-1 (overlaps the DMA stream) ---------------------------
        for ch in range(NCH):
            a_t = ap_.tile([P, 2, N], bf16, tag="a")
            nc.gpsimd.dma_start(out=a_t, in_=adj_c[:, ch, :, :])
            for t2 in range(2):
                r = 2 * ch + t2
                at_r = a_t[:, t2, :]
                d_r = small.tile([P, 1], f32, tag="d")
                junk = scr.tile([P, N], bf16, tag="junk")
                nc.scalar.activation(out=junk, in_=at_r, func=AF.Identity,
                                     accum_out=d_r)
                nc.scalar.activation(out=dsq[:, r:r + 1], in_=d_r, func=AF.Sqrt,
                                     bias=eps_sb)
                nc.vector.reciprocal(out=sinv[:, r:r + 1], in_=dsq[:, r:r + 1])
                diag2 = small.tile([P, P], bf16, tag="diag")
                nc.vector.tensor_scalar(out=diag2, in0=ident_bf,
                                        scalar1=sinv[:, r:r + 1],
                                        scalar2=sinv[:, r:r + 1],
                                        op0=OP.mult, op1=OP.mult)
                nc.vector.tensor_scalar(out=y0n[:, r, :], in0=x_bf[:, r, :],
                                        scalar1=sinv[:, r:r + 1], scalar2=None,
                                        op0=OP.mult)
                # transpose + q[i]-scale A tile row r: 16 (128x128) matmuls
                for g in range(4):
                    pt = ps_t.tile([P, 4, P], f32, tag="t")
                    for qq in range(4):
                        c = 4 * g + qq
                        nc.tensor.matmul(pt[:, qq, :],
                                         lhsT=at_r[:, c * P:(c + 1) * P],
                                         rhs=diag2, start=True, stop=True)
                    nc.vector.tensor_copy(
                        out=ats2[:, 4 * g:4 * g + 4, r * P:(r + 1) * P], in_=pt)

                # transpose y0n tiles into y0T once 4 are ready
                if r % 4 == 3:
                    pt_y0 = ps_t.tile([P, 4, P], f32, tag="t")
                    for qq in range(4):
                        nc.tensor.matmul(pt_y0[:, qq, :], lhsT=y0n[:, r - 3 + qq, :],
                                         rhs=ident_bf, start=True, stop=True)
                    nc.scalar.copy(out=y0T[:, (r - 3) * P:(r + 1) * P],
                                   in_=pt_y0)

                # triangular pass-1 terms that became ready with tile r:
                # (a) column block r, strips c <= r
                # start=True clears has_written for the WHOLE bank (4 column
                # blocks), so only the bank's first-ever matmul may set it;
                # later blocks' first writes overwrite via cleared bits.
                for c in range(r + 1):
                    nc.tensor.matmul(z1[:, r * P:(r + 1) * P], lhsT=y0n[:, c, :],
                                     rhs=ats2[:, c, r * P:(r + 1) * P],
                                     start=(r % 4 == 0 and c == 0),
                                     stop=(r == NT - 1 and c == NT - 1),
                                     skip_group_check=True)
                # (b) new strip r into older column blocks (bank chunks)
                for sg in range((r + 3) // 4):
                    lo = 4 * sg
                    hi = min(lo + 4, r)  # blocks [lo, hi)
                    nc.tensor.matmul(z1[:, lo * P:hi * P], lhsT=y0n[:, r, :],
                                     rhs=ats2[:, r, lo * P:hi * P],
                                     start=False, stop=(r == NT - 1),
                                     skip_group_check=True)

        # ---- y1 from w1 = z1: y1T = y0T - w1T; y1n = y0n - nat(w1T) ----
        for g in range(4):
            alt_copy(w1bf[:, g * 512:(g + 1) * 512], z1[:, g * 512:(g + 1) * 512])
        nc.vector.tensor_tensor(out=y1T, in0=y0T, in1=w1bf, op=OP.subtract)
        for g in range(4):
            pt = ps_t.tile([P, 4, P], f32, tag="t")
            for qq in range(4):
                rr = 4 * g + qq
                nc.tensor.matmul(pt[:, qq, :], lhsT=w1bf[:, rr * P:(rr + 1) * P],
                                 rhs=ident_bf, start=True, stop=True)
            nc.vector.tensor_tensor(out=y1n[:, 4 * g:4 * g + 4, :],
                                    in0=y0n[:, 4 * g:4 * g + 4, :], in1=pt,
                                    op=OP.subtract)
        nc.vector.scalar_tensor_tensor(out=ttT, in0=y1T, scalar=2.0, in1=y0T,
                                       op0=OP.mult, op1=OP.subtract)

        # ---- pass 2 by column group with pipelined epilogue ------------
        z2 = ps_acc.tile([P, N], f32, tag="acc")
        ykT = (y0T, y1T, y2T)
        for g in range(4):
            gl, gh = g * 512, (g + 1) * 512
            for c in range(NT):
                nc.tensor.matmul(z2[:, gl:gh], lhsT=y1n[:, c, :],
                                 rhs=ats2[:, c, gl:gh],
                                 start=(c == 0), stop=(c == NT - 1))
            alt_copy(w2bf[:, gl:gh], z2[:, gl:gh])
            nc.vector.scalar_tensor_tensor(out=y2T[:, gl:gh], in0=w2bf[:, gl:gh],
                                           scalar=-2.0, in1=ttT[:, gl:gh],
                                           op0=OP.mult, op1=OP.add)
            oT = ps_t.tile([P, 512], f32, tag="t")
            for k3 in range(K):
                nc.tensor.matmul(oT, lhsT=w_bf[:, k3, :], rhs=ykT[k3][:, gl:gh],
                                 start=(k3 == 0), stop=(k3 == K - 1))
            alt_copy(oTbf[:, gl:gh], oT)
            on = ps_t.tile([P, 4, P], f32, tag="t")
            for qq in range(4):
                rr = 4 * g + qq
                nc.tensor.matmul(on[:, qq, :], lhsT=oTbf[:, rr * P:(rr + 1) * P],
                                 rhs=ident_bf, start=True, stop=True)
            og = small.tile([P, 4, F], f32, tag="og")
            for qq in range(4):
                rr = 4 * g + qq
                tmp = small.tile([P, F], f32, tag="tmp")
                nc.vector.scalar_tensor_tensor(out=tmp, in0=on[:, qq, :],
                                               scalar=dsq[:, rr:rr + 1], in1=bsum,
                                               op0=OP.mult, op1=OP.add)
                nc.scalar.activation(out=og[:, qq, :], in_=tmp, func=AF.Relu)
            nc.sync.dma_start(out=out_t[:, 4 * g:4 * g + 4, :], in_=og)

    nc.compile()
    return nc


def _get_nc():
    if "nc" not in _cache:
        _cache["nc"] = _build_nc()
    return _cache["nc"]


def make_in_maps(x, adj, W, b):
    ident = np.eye(P, dtype=np.float32)
    x = np.ascontiguousarray(np.asarray(x, dtype=np.float32))
    adj = np.ascontiguousarray(np.asarray(adj, dtype=np.float32))
    Wf = np.ascontiguousarray(np.asarray(W, dtype=np.float32))
    bf = np.asarray(b, dtype=np.float32)
    bsum = np.ascontiguousarray(
        np.broadcast_to(bf.sum(axis=0), (P, F)).astype(np.float32))
    return [
        {"adj": adj[c], "x": x[c], "W": Wf, "bsum": bsum, "ident": ident}
        for c in range(NCORES)
    ]


def run_raw(x, adj, W, b, **kwargs):
    from concourse import bass_utils

    nc = _get_nc()
    in_maps = make_in_maps(x, adj, W, b)
    res = bass_utils.run_bass_kernel_spmd(nc, in_maps,
                                          core_ids=list(range(NCORES)), **kwargs)
    out = np.stack([res.results[c]["out"] for c in range(NCORES)], axis=0)
    return out.astype(np.float32), res


def kernel(x, adj, W, b):
    out, _ = run_raw(x, adj, W, b)
    return out
